# revision 8
# baseline (speedup 1.0000x reference)
# Trainium2 Bass kernel: 2:4 structured activation pruning + Linear.
#
#   out = magnitude_prune_2of4(x.reshape(-1, 4096)) @ weight.T
#
# Sharding: data-parallel over the flattened token dim (16384 tokens ->
# 2048/core across 8 cores); weight replicated. No collectives.
#
# Layout trick: the host casts x to bf16 and permutes the d axis so the
# four group positions land in four contiguous 1024-wide blocks, ordered
# [x0, x2, x1, x3].  Every DVE op in the 2:4 threshold tree then runs on
# packed stride-1 bf16 operands (2x_1p DVE mode), with no strided or
# broadcast access patterns.  The same permutation is applied to the
# weight rows on the host, so the GEMM is unchanged.
#
# Per-core pipeline, per 128-token tile:
#   DMA x (bf16) -> ACT Abs -> DVE stock max/min tree (packed, 2x) ->
#   per-group 2nd-max threshold (exact bf16) -> custom DVE prune
#   (|x| >= thr ? x : 0) -> DMA XBAR transpose SBUF->SBUF (bf16) ->
#   PE bf16 matmul accumulating over 32 d-chunks -> ACT PSUM->SBUF ->
#   DMA out.  The PE does nothing but the GEMM (no on-PE transposes).
import numpy as np

N_CORES = 8
BS, SEQ, D = 4, 4096, 4096
OUTF = 1024
TOK_TOTAL = BS * SEQ
TOK = TOK_TOTAL // N_CORES      # 2048 tokens per core
P = 128                         # SBUF partitions
NT = TOK // P                   # 16 token tiles per core
NCH = D // P                    # 32 d-chunks of 128
G = D // 4                      # 1024 groups per token row

_compiled = None
_custom_ops = None


def _register_custom_dve():
    # Fused DVE prune op: out = |x| >= thr ? x : 0.
    global _custom_ops
    if _custom_ops is not None:
        return _custom_ops
    from concourse import dve_ops as Dv
    from concourse.dve_spec import Spec, Src0, Src1, Zero, maxx, select, lower
    from concourse.dve_uop import DveOpSpec

    def mk(name, body, reference):
        spec = Spec(body=body, reference=reference)
        shas = {}
        for ver in ("v3", "v4"):
            try:
                u = lower(spec, ver=ver)
                shas[ver] = DveOpSpec(name=name, opcode=1, uops=u,
                                      rd1_en=True).sha(ver)
            except Exception:
                if ver == "v3":
                    raise
        return Dv.DveOp(name=name, spec=spec, subdim=False, uops_sha=shas)

    ops = (
        mk("PRUNE24_ANT", select(maxx(Src0, Zero - Src0) >= Src1, Src0, Zero),
           lambda in0, in1: np.where(np.abs(in0) >= in1, in0, 0.0)),
    )
    for op in ops:
        if op.name not in Dv._SUB_OPCODE_FOR_NAME:
            Dv.OPS.append(op)
            Dv.CUSTOM_DVE_SPECS[op.name] = op.spec
            Dv._SUB_OPCODE_FOR_NAME[op.name] = (
                Dv._CUSTOM_DVE_ROW_BASE + len(Dv._SUB_OPCODE_FOR_NAME))
    _custom_ops = ops
    return ops


def _build():
    import concourse.tile as tile
    import concourse.mybir as mybir
    from concourse import bacc

    (PRUNE24,) = _register_custom_dve()
    f32 = mybir.dt.float32
    bf16 = mybir.dt.bfloat16
    Alu = mybir.AluOpType
    Act = mybir.ActivationFunctionType

    nc = bacc.Bacc("TRN2", target_bir_lowering=False, debug=False,
                   num_devices=N_CORES)
    xs_ap = nc.dram_tensor("xs", [TOK, D], bf16, kind="ExternalInput").ap()
    wt_ap = nc.dram_tensor("wt", [D, OUTF], bf16, kind="ExternalInput").ap()
    o_ap = nc.dram_tensor("o", [TOK, OUTF], f32, kind="ExternalOutput").ap()

    with tile.TileContext(nc) as tc:
        with tc.tile_pool(name="wpool", bufs=1) as wpool, \
             tc.tile_pool(name="xin", bufs=2) as xin, \
             tc.tile_pool(name="awork", bufs=2) as awork, \
             tc.tile_pool(name="mwork", bufs=2) as mwork, \
             tc.tile_pool(name="xsp_p", bufs=2) as xsp_p, \
             tc.tile_pool(name="xtp", bufs=3) as xtp, \
             tc.tile_pool(name="outp", bufs=2) as outp, \
             tc.tile_pool(name="pso", bufs=6, space="PSUM") as pso:

            # weight.T (host-permuted) resident in SBUF:
            # [d-in-chunk partitions, chunk, outf].  8 HWDGE DMAs on the ACT
            # queue: early chunks land first (matmul chunk c only waits on
            # its own block), and no gpsimd SWDGE drain sits in the critical
            # path of the first tile.
            w_sb = wpool.tile([P, NCH, OUTF], bf16)
            WB = NCH // 8
            for k in range(8):
                nc.scalar.dma_start(
                    out=w_sb[:, k * WB:(k + 1) * WB, :],
                    in_=wt_ap[k * WB * P:(k + 1) * WB * P, :]
                    .rearrange("(c p) f -> p c f", p=P))

            for i in range(NT):
                xh = xin.tile([P, D], bf16, tag="xh")
                nc.sync.dma_start(out=xh, in_=xs_ap[i * P:(i + 1) * P, :])
                # |x| on ACT; the threshold tree reads only this
                ab = awork.tile([P, D], bf16, tag="ab")
                nc.scalar.activation(ab, xh, Act.Abs)
                # block order is [x0, x2, x1, x3]: halves pair (x0,x1) and
                # (x2,x3) elementwise -> all packed stride-1 DVE ops
                mx = mwork.tile([P, 2 * G], bf16, tag="mx")
                mn = mwork.tile([P, 2 * G], bf16, tag="mn")
                nc.vector.tensor_tensor(mx, ab[:, :2 * G], ab[:, 2 * G:],
                                        Alu.max)
                nc.vector.tensor_tensor(mn, ab[:, :2 * G], ab[:, 2 * G:],
                                        Alu.min)
                # thr = 2nd-largest |x| per group = max(min(maxes), max(mins))
                t1 = mwork.tile([P, G], bf16, tag="t1")
                t2 = mwork.tile([P, G], bf16, tag="t2")
                nc.vector.tensor_tensor(t1, mx[:, :G], mx[:, G:], Alu.min)
                nc.vector.tensor_tensor(t2, mn[:, :G], mn[:, G:], Alu.max)
                nc.vector.tensor_tensor(t1, t1, t2, Alu.max)
                # prune: xsp = |x| >= thr ? x : 0 (exact bf16 compare)
                xsp = xsp_p.tile([P, D], bf16, tag="xsp")
                thr_b = t1.unsqueeze(1).broadcast_to([P, 4, G])
                nc.vector._custom_dve(
                    PRUNE24,
                    out=xsp.rearrange("p (j g) -> p j g", j=4),
                    in0=xh.rearrange("p (j g) -> p j g", j=4),
                    in1=thr_b)
                # transpose [tok, d] -> [d%128, d//128, tok] on the DMA XBAR,
                # split in two halves on separate queues so dispatch overlaps
                # and the first matmuls only wait on the first half
                xspT = xtp.tile([P, NCH, P], bf16, tag="xspT")
                nc.sync.dma_start(out=xspT[:, :NCH // 2, :],
                                  in_=xsp[:, :D // 2], transpose=True)
                nc.scalar.dma_start(out=xspT[:, NCH // 2:, :],
                                    in_=xsp[:, D // 2:], transpose=True)
                # matmul: psum[tok, outf-half] += xspT[c].T @ wT[c]
                for n in range(2):
                    pout = pso.tile([P, OUTF // 2], f32)
                    for c in range(NCH):
                        nc.tensor.matmul(pout,
                                         xspT[:, c, :],
                                         w_sb[:, c, n * 512:(n + 1) * 512],
                                         start=(c == 0), stop=(c == NCH - 1))
                    osb = outp.tile([P, OUTF // 2], f32, tag="osb")
                    nc.scalar.copy(osb, pout)
                    nc.sync.dma_start(
                        out=o_ap[i * P:(i + 1) * P, n * 512:(n + 1) * 512],
                        in_=osb)
    nc.compile()
    return nc


def _get_compiled():
    global _compiled
    if _compiled is None:
        _compiled = _build()
    return _compiled


def _fix_ties_bf16(xb32, x_orig):
    # The device keeps elements with |x| >= (2nd-largest |x| of the group),
    # evaluated on bf16 values.  When the 2nd and 3rd magnitudes round to
    # the same bf16 the device would keep 3+ elements, while the reference
    # (fp32 top_k, stable) keeps exactly 2.  Pre-zero the reference-dropped
    # elements of tied groups so the device selection matches exactly.
    g = np.abs(xb32.reshape(-1, 4))
    s = np.sort(g, axis=1)          # ascending: s[:,2]=2nd largest, s[:,1]=3rd
    tied = s[:, 2] == s[:, 1]
    if not tied.any():
        return xb32
    gv = xb32.reshape(-1, 4)
    go = x_orig.reshape(-1, 4)[tied]
    keep = np.argsort(-np.abs(go), axis=1, kind="stable")[:, :2]
    mask = np.zeros(go.shape, dtype=bool)
    np.put_along_axis(mask, keep, True, axis=1)
    sub = gv[tied]
    sub[~mask] = 0.0
    gv[tied] = sub
    return xb32


# group-position block order: pairs (x0,x1) and (x2,x3) land in opposite
# halves so the DVE tree pairs them with packed stride-1 slices
_BLOCK_ORDER = [0, 2, 1, 3]


def kernel(x: np.ndarray, weight: np.ndarray) -> np.ndarray:
    import ml_dtypes
    from concourse.bass_utils import run_bass_kernel_spmd

    nc = _get_compiled()
    bo = _BLOCK_ORDER
    x_flat = np.ascontiguousarray(x.reshape(TOK_TOTAL, D), dtype=np.float32)
    xb32 = x_flat.astype(ml_dtypes.bfloat16).astype(np.float32)
    xb32 = _fix_ties_bf16(xb32, x_flat)
    # permute d so group position j sits in contiguous block bo.index(j)
    xp = np.ascontiguousarray(
        xb32.reshape(TOK_TOTAL, G, 4)[:, :, bo].transpose(0, 2, 1)
        .reshape(TOK_TOTAL, D)).astype(ml_dtypes.bfloat16)
    wtp = np.ascontiguousarray(
        weight.T.astype(np.float32).reshape(G, 4, OUTF)[:, bo, :]
        .transpose(1, 0, 2).reshape(D, OUTF)).astype(ml_dtypes.bfloat16)
    in_maps = [{"xs": xp[c * TOK:(c + 1) * TOK], "wt": wtp}
               for c in range(N_CORES)]
    res = run_bass_kernel_spmd(nc, in_maps, core_ids=list(range(N_CORES)))
    out = np.concatenate([res.results[c]["o"] for c in range(N_CORES)], axis=0)
    return out.reshape(BS, SEQ, OUTF)


# revision 9
# speedup vs baseline: 1.0062x; 1.0062x over previous
# Trainium2 Bass kernel: 2:4 structured activation pruning + Linear.
#
#   out = magnitude_prune_2of4(x.reshape(-1, 4096)) @ weight.T
#
# Sharding: data-parallel over the flattened token dim (16384 tokens ->
# 2048/core across 8 cores); weight replicated. No collectives.
#
# Layout trick: the host casts x to bf16 and permutes the d axis so the
# four group positions land in four contiguous 1024-wide blocks, ordered
# [x0, x2, x1, x3].  Every DVE op in the 2:4 threshold tree then runs on
# packed stride-1 bf16 operands (2x_1p DVE mode), with no strided or
# broadcast access patterns.  The same permutation is applied to the
# weight rows on the host, so the GEMM is unchanged.
#
# Per-core pipeline, per 128-token tile:
#   DMA x (bf16) -> ACT Abs -> DVE stock max/min tree (packed, 2x) ->
#   per-group 2nd-max threshold (exact bf16) -> custom DVE prune
#   (|x| >= thr ? x : 0) -> DMA XBAR transpose SBUF->SBUF (bf16) ->
#   PE bf16 matmul accumulating over 32 d-chunks -> ACT PSUM->SBUF ->
#   DMA out.  The PE does nothing but the GEMM (no on-PE transposes).
import numpy as np

N_CORES = 8
BS, SEQ, D = 4, 4096, 4096
OUTF = 1024
TOK_TOTAL = BS * SEQ
TOK = TOK_TOTAL // N_CORES      # 2048 tokens per core
P = 128                         # SBUF partitions
NT = TOK // P                   # 16 token tiles per core
NCH = D // P                    # 32 d-chunks of 128
G = D // 4                      # 1024 groups per token row

_compiled = None
_custom_ops = None


def _register_custom_dve():
    # Fused DVE prune op: out = |x| >= thr ? x : 0.
    global _custom_ops
    if _custom_ops is not None:
        return _custom_ops
    from concourse import dve_ops as Dv
    from concourse.dve_spec import Spec, Src0, Src1, Zero, maxx, select, lower
    from concourse.dve_uop import DveOpSpec

    def mk(name, body, reference):
        spec = Spec(body=body, reference=reference)
        shas = {}
        for ver in ("v3", "v4"):
            try:
                u = lower(spec, ver=ver)
                shas[ver] = DveOpSpec(name=name, opcode=1, uops=u,
                                      rd1_en=True).sha(ver)
            except Exception:
                if ver == "v3":
                    raise
        return Dv.DveOp(name=name, spec=spec, subdim=False, uops_sha=shas)

    ops = (
        mk("PRUNE24_ANT", select(maxx(Src0, Zero - Src0) >= Src1, Src0, Zero),
           lambda in0, in1: np.where(np.abs(in0) >= in1, in0, 0.0)),
    )
    for op in ops:
        if op.name not in Dv._SUB_OPCODE_FOR_NAME:
            Dv.OPS.append(op)
            Dv.CUSTOM_DVE_SPECS[op.name] = op.spec
            Dv._SUB_OPCODE_FOR_NAME[op.name] = (
                Dv._CUSTOM_DVE_ROW_BASE + len(Dv._SUB_OPCODE_FOR_NAME))
    _custom_ops = ops
    return ops


def _build():
    import concourse.tile as tile
    import concourse.mybir as mybir
    from concourse import bacc

    (PRUNE24,) = _register_custom_dve()
    f32 = mybir.dt.float32
    bf16 = mybir.dt.bfloat16
    Alu = mybir.AluOpType
    Act = mybir.ActivationFunctionType

    nc = bacc.Bacc("TRN2", target_bir_lowering=False, debug=False,
                   num_devices=N_CORES)
    xs_ap = nc.dram_tensor("xs", [TOK, D], bf16, kind="ExternalInput").ap()
    wt_ap = nc.dram_tensor("wt", [D, OUTF], bf16, kind="ExternalInput").ap()
    o_ap = nc.dram_tensor("o", [TOK, OUTF], f32, kind="ExternalOutput").ap()

    with tile.TileContext(nc) as tc:
        with tc.tile_pool(name="wpool", bufs=1) as wpool, \
             tc.tile_pool(name="xin", bufs=2) as xin, \
             tc.tile_pool(name="awork", bufs=2) as awork, \
             tc.tile_pool(name="mwork", bufs=2) as mwork, \
             tc.tile_pool(name="xsp_p", bufs=2) as xsp_p, \
             tc.tile_pool(name="xtp", bufs=3) as xtp, \
             tc.tile_pool(name="outp", bufs=2) as outp, \
             tc.tile_pool(name="pso", bufs=6, space="PSUM") as pso:

            # weight.T (host-permuted) resident in SBUF:
            # [d-in-chunk partitions, chunk, outf].  Loaded via gpsimd SWDGE
            # lanes: keeps the 8MB transfer off both HWDGE FIFO rings (SP and
            # ACT), which the x loads and XBAR transposes need low-latency.
            w_sb = wpool.tile([P, NCH, OUTF], bf16)
            WB = NCH // 4
            for k in range(4):
                nc.gpsimd.dma_start(
                    out=w_sb[:, k * WB:(k + 1) * WB, :],
                    in_=wt_ap[k * WB * P:(k + 1) * WB * P, :]
                    .rearrange("(c p) f -> p c f", p=P))

            for i in range(NT):
                xh = xin.tile([P, D], bf16, tag="xh")
                nc.sync.dma_start(out=xh, in_=xs_ap[i * P:(i + 1) * P, :])
                # |x| on ACT; the threshold tree reads only this
                ab = awork.tile([P, D], bf16, tag="ab")
                nc.scalar.activation(ab, xh, Act.Abs)
                # block order is [x0, x2, x1, x3]: halves pair (x0,x1) and
                # (x2,x3) elementwise -> all packed stride-1 DVE ops
                mx = mwork.tile([P, 2 * G], bf16, tag="mx")
                mn = mwork.tile([P, 2 * G], bf16, tag="mn")
                nc.vector.tensor_tensor(mx, ab[:, :2 * G], ab[:, 2 * G:],
                                        Alu.max)
                nc.vector.tensor_tensor(mn, ab[:, :2 * G], ab[:, 2 * G:],
                                        Alu.min)
                # thr = 2nd-largest |x| per group = max(min(maxes), max(mins))
                t1 = mwork.tile([P, G], bf16, tag="t1")
                t2 = mwork.tile([P, G], bf16, tag="t2")
                nc.vector.tensor_tensor(t1, mx[:, :G], mx[:, G:], Alu.min)
                nc.vector.tensor_tensor(t2, mn[:, :G], mn[:, G:], Alu.max)
                nc.vector.tensor_tensor(t1, t1, t2, Alu.max)
                # prune: xsp = |x| >= thr ? x : 0 (exact bf16 compare)
                xsp = xsp_p.tile([P, D], bf16, tag="xsp")
                thr_b = t1.unsqueeze(1).broadcast_to([P, 4, G])
                nc.vector._custom_dve(
                    PRUNE24,
                    out=xsp.rearrange("p (j g) -> p j g", j=4),
                    in0=xh.rearrange("p (j g) -> p j g", j=4),
                    in1=thr_b)
                # transpose [tok, d] -> [d%128, d//128, tok] on the DMA XBAR,
                # split in two halves on separate queues so dispatch overlaps
                # and the first matmuls only wait on the first half
                xspT = xtp.tile([P, NCH, P], bf16, tag="xspT")
                nc.sync.dma_start(out=xspT[:, :NCH // 2, :],
                                  in_=xsp[:, :D // 2], transpose=True)
                nc.scalar.dma_start(out=xspT[:, NCH // 2:, :],
                                    in_=xsp[:, D // 2:], transpose=True)
                # matmul: psum[tok, outf-half] += xspT[c].T @ wT[c]
                for n in range(2):
                    pout = pso.tile([P, OUTF // 2], f32)
                    for c in range(NCH):
                        nc.tensor.matmul(pout,
                                         xspT[:, c, :],
                                         w_sb[:, c, n * 512:(n + 1) * 512],
                                         start=(c == 0), stop=(c == NCH - 1))
                    osb = outp.tile([P, OUTF // 2], f32, tag="osb")
                    nc.scalar.copy(osb, pout)
                    nc.sync.dma_start(
                        out=o_ap[i * P:(i + 1) * P, n * 512:(n + 1) * 512],
                        in_=osb)
    nc.compile()
    return nc


def _get_compiled():
    global _compiled
    if _compiled is None:
        _compiled = _build()
    return _compiled


def _fix_ties_bf16(xb32, x_orig):
    # The device keeps elements with |x| >= (2nd-largest |x| of the group),
    # evaluated on bf16 values.  When the 2nd and 3rd magnitudes round to
    # the same bf16 the device would keep 3+ elements, while the reference
    # (fp32 top_k, stable) keeps exactly 2.  Pre-zero the reference-dropped
    # elements of tied groups so the device selection matches exactly.
    g = np.abs(xb32.reshape(-1, 4))
    s = np.sort(g, axis=1)          # ascending: s[:,2]=2nd largest, s[:,1]=3rd
    tied = s[:, 2] == s[:, 1]
    if not tied.any():
        return xb32
    gv = xb32.reshape(-1, 4)
    go = x_orig.reshape(-1, 4)[tied]
    keep = np.argsort(-np.abs(go), axis=1, kind="stable")[:, :2]
    mask = np.zeros(go.shape, dtype=bool)
    np.put_along_axis(mask, keep, True, axis=1)
    sub = gv[tied]
    sub[~mask] = 0.0
    gv[tied] = sub
    return xb32


# group-position block order: pairs (x0,x1) and (x2,x3) land in opposite
# halves so the DVE tree pairs them with packed stride-1 slices
_BLOCK_ORDER = [0, 2, 1, 3]


def kernel(x: np.ndarray, weight: np.ndarray) -> np.ndarray:
    import ml_dtypes
    from concourse.bass_utils import run_bass_kernel_spmd

    nc = _get_compiled()
    bo = _BLOCK_ORDER
    x_flat = np.ascontiguousarray(x.reshape(TOK_TOTAL, D), dtype=np.float32)
    xb32 = x_flat.astype(ml_dtypes.bfloat16).astype(np.float32)
    xb32 = _fix_ties_bf16(xb32, x_flat)
    # permute d so group position j sits in contiguous block bo.index(j)
    xp = np.ascontiguousarray(
        xb32.reshape(TOK_TOTAL, G, 4)[:, :, bo].transpose(0, 2, 1)
        .reshape(TOK_TOTAL, D)).astype(ml_dtypes.bfloat16)
    wtp = np.ascontiguousarray(
        weight.T.astype(np.float32).reshape(G, 4, OUTF)[:, bo, :]
        .transpose(1, 0, 2).reshape(D, OUTF)).astype(ml_dtypes.bfloat16)
    in_maps = [{"xs": xp[c * TOK:(c + 1) * TOK], "wt": wtp}
               for c in range(N_CORES)]
    res = run_bass_kernel_spmd(nc, in_maps, core_ids=list(range(N_CORES)))
    out = np.concatenate([res.results[c]["o"] for c in range(N_CORES)], axis=0)
    return out.reshape(BS, SEQ, OUTF)


# revision 12
# speedup vs baseline: 1.0858x; 1.0791x over previous
# Trainium2 Bass kernel: 2:4 structured activation pruning + Linear.
#
#   out = magnitude_prune_2of4(x.reshape(-1, 4096)) @ weight.T
#
# Sharding: data-parallel over the flattened token dim (16384 tokens ->
# 2048/core across 8 cores); weight replicated. No collectives.
#
# Layout trick: the host casts x to bf16 and permutes the d axis so the
# four group positions land in four contiguous 1024-wide blocks, ordered
# [x0, x2, x1, x3].  Every DVE op in the 2:4 threshold tree then runs on
# packed stride-1 bf16 operands (2x_1p DVE mode), with no strided or
# broadcast access patterns.  The same permutation is applied to the
# weight rows on the host, so the GEMM is unchanged.
#
# Per-core pipeline, per 128-token tile:
#   DMA x (bf16) -> ACT Abs -> DVE stock max/min tree (packed, 2x) ->
#   per-group 2nd-max threshold (exact bf16) -> custom DVE prune
#   (|x| >= thr ? x : 0) -> DMA XBAR transpose SBUF->SBUF (bf16) ->
#   PE bf16 matmul accumulating over 32 d-chunks -> ACT PSUM->SBUF ->
#   DMA out.  The PE does nothing but the GEMM (no on-PE transposes).
import numpy as np

N_CORES = 8
BS, SEQ, D = 4, 4096, 4096
OUTF = 1024
TOK_TOTAL = BS * SEQ
TOK = TOK_TOTAL // N_CORES      # 2048 tokens per core
P = 128                         # SBUF partitions
NT = TOK // P                   # 16 token tiles per core
NCH = D // P                    # 32 d-chunks of 128
G = D // 4                      # 1024 groups per token row

_compiled = None
_custom_ops = None


def _register_custom_dve():
    # Fused DVE prune op: out = |x| >= thr ? x : 0.
    global _custom_ops
    if _custom_ops is not None:
        return _custom_ops
    from concourse import dve_ops as Dv
    from concourse.dve_spec import Spec, Src0, Src1, Zero, maxx, select, lower
    from concourse.dve_uop import DveOpSpec

    def mk(name, body, reference):
        spec = Spec(body=body, reference=reference)
        shas = {}
        for ver in ("v3", "v4"):
            try:
                u = lower(spec, ver=ver)
                shas[ver] = DveOpSpec(name=name, opcode=1, uops=u,
                                      rd1_en=True).sha(ver)
            except Exception:
                if ver == "v3":
                    raise
        return Dv.DveOp(name=name, spec=spec, subdim=False, uops_sha=shas)

    ops = (
        mk("PRUNE24_ANT", select(maxx(Src0, Zero - Src0) >= Src1, Src0, Zero),
           lambda in0, in1: np.where(np.abs(in0) >= in1, in0, 0.0)),
    )
    for op in ops:
        if op.name not in Dv._SUB_OPCODE_FOR_NAME:
            Dv.OPS.append(op)
            Dv.CUSTOM_DVE_SPECS[op.name] = op.spec
            Dv._SUB_OPCODE_FOR_NAME[op.name] = (
                Dv._CUSTOM_DVE_ROW_BASE + len(Dv._SUB_OPCODE_FOR_NAME))
    _custom_ops = ops
    return ops


def _build():
    import concourse.tile as tile
    import concourse.mybir as mybir
    from concourse import bacc

    (PRUNE24,) = _register_custom_dve()
    f32 = mybir.dt.float32
    bf16 = mybir.dt.bfloat16
    Alu = mybir.AluOpType
    Act = mybir.ActivationFunctionType

    nc = bacc.Bacc("TRN2", target_bir_lowering=False, debug=False,
                   num_devices=N_CORES)
    xs_ap = nc.dram_tensor("xs", [TOK, D], bf16, kind="ExternalInput").ap()
    wt_ap = nc.dram_tensor("wt", [D, OUTF], bf16, kind="ExternalInput").ap()
    o_ap = nc.dram_tensor("o", [TOK, OUTF], f32, kind="ExternalOutput").ap()

    with tile.TileContext(nc) as tc:
        with tc.tile_pool(name="wpool", bufs=1) as wpool, \
             tc.tile_pool(name="xin", bufs=3) as xin, \
             tc.tile_pool(name="awork", bufs=2) as awork, \
             tc.tile_pool(name="mwork", bufs=2) as mwork, \
             tc.tile_pool(name="xsp_p", bufs=3) as xsp_p, \
             tc.tile_pool(name="xtp", bufs=3) as xtp, \
             tc.tile_pool(name="outp", bufs=2) as outp, \
             tc.tile_pool(name="pso", bufs=6, space="PSUM") as pso:

            # weight.T (host-permuted) resident in SBUF:
            # [d-in-chunk partitions, chunk, outf].  Loaded via gpsimd SWDGE
            # lanes: keeps the 8MB transfer off both HWDGE FIFO rings (SP and
            # ACT), which the XBAR transposes need low-latency.  Split by
            # outf-half first: tile 0's n=0 matmuls only need the first 4MB.
            w_sb = wpool.tile([P, NCH, OUTF], bf16)
            WB = NCH // 4
            for n in range(2):
                for k in range(4):
                    nc.gpsimd.dma_start(
                        out=w_sb[:, k * WB:(k + 1) * WB,
                                 n * 512:(n + 1) * 512],
                        in_=wt_ap[k * WB * P:(k + 1) * WB * P,
                                  n * 512:(n + 1) * 512]
                        .rearrange("(c p) f -> p c f", p=P))

            for i in range(NT):
                # x loads ride the ACT HWDGE ring; the SP ring keeps only
                # transpose-half-A + output stores so transposes start fast
                xh = xin.tile([P, D], bf16, tag="xh")
                nc.scalar.dma_start(out=xh, in_=xs_ap[i * P:(i + 1) * P, :])
                # |x| on ACT; the threshold tree reads only this
                ab = awork.tile([P, D], bf16, tag="ab")
                nc.scalar.activation(ab, xh, Act.Abs)
                # block order is [x0, x2, x1, x3]: halves pair (x0,x1) and
                # (x2,x3) elementwise -> all packed stride-1 DVE ops
                mx = mwork.tile([P, 2 * G], bf16, tag="mx")
                mn = mwork.tile([P, 2 * G], bf16, tag="mn")
                nc.vector.tensor_tensor(mx, ab[:, :2 * G], ab[:, 2 * G:],
                                        Alu.max)
                nc.vector.tensor_tensor(mn, ab[:, :2 * G], ab[:, 2 * G:],
                                        Alu.min)
                # thr = 2nd-largest |x| per group = max(min(maxes), max(mins))
                t1 = mwork.tile([P, G], bf16, tag="t1")
                t2 = mwork.tile([P, G], bf16, tag="t2")
                nc.vector.tensor_tensor(t1, mx[:, :G], mx[:, G:], Alu.min)
                nc.vector.tensor_tensor(t2, mn[:, :G], mn[:, G:], Alu.max)
                nc.vector.tensor_tensor(t1, t1, t2, Alu.max)
                # prune: xsp = |x| >= thr ? x : 0 (exact bf16 compare)
                xsp = xsp_p.tile([P, D], bf16, tag="xsp")
                thr_b = t1.unsqueeze(1).broadcast_to([P, 4, G])
                nc.vector._custom_dve(
                    PRUNE24,
                    out=xsp.rearrange("p (j g) -> p j g", j=4),
                    in0=xh.rearrange("p (j g) -> p j g", j=4),
                    in1=thr_b)
                # transpose [tok, d] -> [d%128, d//128, tok] on the DMA XBAR,
                # split in two halves on separate queues so dispatch overlaps
                # and the first matmuls only wait on the first half
                xspT = xtp.tile([P, NCH, P], bf16, tag="xspT")
                nc.sync.dma_start(out=xspT[:, :NCH // 2, :],
                                  in_=xsp[:, :D // 2], transpose=True)
                nc.scalar.dma_start(out=xspT[:, NCH // 2:, :],
                                    in_=xsp[:, D // 2:], transpose=True)
                # matmul: psum[tok, outf-half] += xspT[c].T @ wT[c]
                for n in range(2):
                    pout = pso.tile([P, OUTF // 2], f32)
                    for c in range(NCH):
                        nc.tensor.matmul(pout,
                                         xspT[:, c, :],
                                         w_sb[:, c, n * 512:(n + 1) * 512],
                                         start=(c == 0), stop=(c == NCH - 1))
                    osb = outp.tile([P, OUTF // 2], f32, tag="osb")
                    nc.scalar.copy(osb, pout)
                    nc.sync.dma_start(
                        out=o_ap[i * P:(i + 1) * P, n * 512:(n + 1) * 512],
                        in_=osb)
    nc.compile()
    return nc


def _get_compiled():
    global _compiled
    if _compiled is None:
        _compiled = _build()
    return _compiled


def _fix_ties_bf16(xb32, x_orig):
    # The device keeps elements with |x| >= (2nd-largest |x| of the group),
    # evaluated on bf16 values.  When the 2nd and 3rd magnitudes round to
    # the same bf16 the device would keep 3+ elements, while the reference
    # (fp32 top_k, stable) keeps exactly 2.  Pre-zero the reference-dropped
    # elements of tied groups so the device selection matches exactly.
    g = np.abs(xb32.reshape(-1, 4))
    s = np.sort(g, axis=1)          # ascending: s[:,2]=2nd largest, s[:,1]=3rd
    tied = s[:, 2] == s[:, 1]
    if not tied.any():
        return xb32
    gv = xb32.reshape(-1, 4)
    go = x_orig.reshape(-1, 4)[tied]
    keep = np.argsort(-np.abs(go), axis=1, kind="stable")[:, :2]
    mask = np.zeros(go.shape, dtype=bool)
    np.put_along_axis(mask, keep, True, axis=1)
    sub = gv[tied]
    sub[~mask] = 0.0
    gv[tied] = sub
    return xb32


# group-position block order: pairs (x0,x1) and (x2,x3) land in opposite
# halves so the DVE tree pairs them with packed stride-1 slices
_BLOCK_ORDER = [0, 2, 1, 3]


def kernel(x: np.ndarray, weight: np.ndarray) -> np.ndarray:
    import ml_dtypes
    from concourse.bass_utils import run_bass_kernel_spmd

    nc = _get_compiled()
    bo = _BLOCK_ORDER
    x_flat = np.ascontiguousarray(x.reshape(TOK_TOTAL, D), dtype=np.float32)
    xb32 = x_flat.astype(ml_dtypes.bfloat16).astype(np.float32)
    xb32 = _fix_ties_bf16(xb32, x_flat)
    # permute d so group position j sits in contiguous block bo.index(j)
    xp = np.ascontiguousarray(
        xb32.reshape(TOK_TOTAL, G, 4)[:, :, bo].transpose(0, 2, 1)
        .reshape(TOK_TOTAL, D)).astype(ml_dtypes.bfloat16)
    wtp = np.ascontiguousarray(
        weight.T.astype(np.float32).reshape(G, 4, OUTF)[:, bo, :]
        .transpose(1, 0, 2).reshape(D, OUTF)).astype(ml_dtypes.bfloat16)
    in_maps = [{"xs": xp[c * TOK:(c + 1) * TOK], "wt": wtp}
               for c in range(N_CORES)]
    res = run_bass_kernel_spmd(nc, in_maps, core_ids=list(range(N_CORES)))
    out = np.concatenate([res.results[c]["o"] for c in range(N_CORES)], axis=0)
    return out.reshape(BS, SEQ, OUTF)


# revision 14
# speedup vs baseline: 1.0962x; 1.0096x over previous
# Trainium2 Bass kernel: 2:4 structured activation pruning + Linear.
#
#   out = magnitude_prune_2of4(x.reshape(-1, 4096)) @ weight.T
#
# Sharding: data-parallel over the flattened token dim (16384 tokens ->
# 2048/core across 8 cores); weight replicated. No collectives.
#
# Layout trick: the host casts x to bf16 and permutes the d axis so the
# four group positions land in four contiguous 1024-wide blocks, ordered
# [x0, x2, x1, x3].  Every DVE op in the 2:4 threshold tree then runs on
# packed stride-1 bf16 operands (2x_1p DVE mode), with no strided or
# broadcast access patterns.  The same permutation is applied to the
# weight rows on the host, so the GEMM is unchanged.
#
# Per-core pipeline, per 128-token tile:
#   DMA x (bf16) -> ACT Abs -> DVE stock max/min tree (packed, 2x) ->
#   per-group 2nd-max threshold (exact bf16) -> custom DVE prune
#   (|x| >= thr ? x : 0) -> DMA XBAR transpose SBUF->SBUF (bf16) ->
#   PE bf16 matmul accumulating over 32 d-chunks -> ACT PSUM->SBUF ->
#   DMA out.  The PE does nothing but the GEMM (no on-PE transposes).
import numpy as np

N_CORES = 8
BS, SEQ, D = 4, 4096, 4096
OUTF = 1024
TOK_TOTAL = BS * SEQ
TOK = TOK_TOTAL // N_CORES      # 2048 tokens per core
P = 128                         # SBUF partitions
NT = TOK // P                   # 16 token tiles per core
NCH = D // P                    # 32 d-chunks of 128
G = D // 4                      # 1024 groups per token row

_compiled = None
_custom_ops = None


def _register_custom_dve():
    # Fused DVE prune op: out = |x| >= thr ? x : 0.
    global _custom_ops
    if _custom_ops is not None:
        return _custom_ops
    from concourse import dve_ops as Dv
    from concourse.dve_spec import Spec, Src0, Src1, Zero, maxx, select, lower
    from concourse.dve_uop import DveOpSpec

    def mk(name, body, reference):
        spec = Spec(body=body, reference=reference)
        shas = {}
        for ver in ("v3", "v4"):
            try:
                u = lower(spec, ver=ver)
                shas[ver] = DveOpSpec(name=name, opcode=1, uops=u,
                                      rd1_en=True).sha(ver)
            except Exception:
                if ver == "v3":
                    raise
        return Dv.DveOp(name=name, spec=spec, subdim=False, uops_sha=shas)

    ops = (
        mk("PRUNE24_ANT", select(maxx(Src0, Zero - Src0) >= Src1, Src0, Zero),
           lambda in0, in1: np.where(np.abs(in0) >= in1, in0, 0.0)),
    )
    for op in ops:
        if op.name not in Dv._SUB_OPCODE_FOR_NAME:
            Dv.OPS.append(op)
            Dv.CUSTOM_DVE_SPECS[op.name] = op.spec
            Dv._SUB_OPCODE_FOR_NAME[op.name] = (
                Dv._CUSTOM_DVE_ROW_BASE + len(Dv._SUB_OPCODE_FOR_NAME))
    _custom_ops = ops
    return ops


def _build():
    import concourse.tile as tile
    import concourse.mybir as mybir
    from concourse import bacc

    (PRUNE24,) = _register_custom_dve()
    f32 = mybir.dt.float32
    bf16 = mybir.dt.bfloat16
    Alu = mybir.AluOpType
    Act = mybir.ActivationFunctionType

    nc = bacc.Bacc("TRN2", target_bir_lowering=False, debug=False,
                   num_devices=N_CORES)
    xs_ap = nc.dram_tensor("xs", [TOK, D], bf16, kind="ExternalInput").ap()
    wt_ap = nc.dram_tensor("wt", [D, OUTF], bf16, kind="ExternalInput").ap()
    o_ap = nc.dram_tensor("o", [TOK, OUTF], f32, kind="ExternalOutput").ap()

    with tile.TileContext(nc) as tc:
        with tc.tile_pool(name="wpool", bufs=1) as wpool, \
             tc.tile_pool(name="xin", bufs=3) as xin, \
             tc.tile_pool(name="awork", bufs=2) as awork, \
             tc.tile_pool(name="mwork", bufs=2) as mwork, \
             tc.tile_pool(name="xsp_p", bufs=3) as xsp_p, \
             tc.tile_pool(name="xtp", bufs=3) as xtp, \
             tc.tile_pool(name="outp", bufs=2) as outp, \
             tc.tile_pool(name="pso", bufs=6, space="PSUM") as pso:

            # weight.T (host-permuted) resident in SBUF:
            # [d-in-chunk partitions, chunk, outf].  outf-half 0 (4MB, all
            # tile 0's n=0 matmuls need) via gpsimd SWDGE, which drains
            # before the first XBAR transpose is ready (the transposes
            # serialize behind the SWDGE queue).  outf-half 1 rides the two
            # HWDGE rings as 1MB pieces interleaved after the first tiles'
            # transposes (emitted inside the tile loop below).
            w_sb = wpool.tile([P, NCH, OUTF], bf16)
            WB = NCH // 4
            for k in range(4):
                nc.gpsimd.dma_start(
                    out=w_sb[:, k * WB:(k + 1) * WB, 0:512],
                    in_=wt_ap[k * WB * P:(k + 1) * WB * P, 0:512]
                    .rearrange("(c p) f -> p c f", p=P))

            def load_w_h1_piece(eng, j):
                # piece j: chunks 8j..8j+7 of outf half 1
                eng.dma_start(
                    out=w_sb[:, j * 8:(j + 1) * 8, 512:1024],
                    in_=wt_ap[j * 8 * P:(j + 1) * 8 * P, 512:1024]
                    .rearrange("(c p) f -> p c f", p=P))

            for i in range(NT):
                # x loads ride the ACT HWDGE ring; the SP ring keeps only
                # transpose-half-A + output stores so transposes start fast
                xh = xin.tile([P, D], bf16, tag="xh")
                nc.scalar.dma_start(out=xh, in_=xs_ap[i * P:(i + 1) * P, :])
                # |x| on ACT; the threshold tree reads only this
                ab = awork.tile([P, D], bf16, tag="ab")
                nc.scalar.activation(ab, xh, Act.Abs)
                # block order is [x0, x2, x1, x3]: halves pair (x0,x1) and
                # (x2,x3) elementwise -> all packed stride-1 DVE ops
                mx = mwork.tile([P, 2 * G], bf16, tag="mx")
                mn = mwork.tile([P, 2 * G], bf16, tag="mn")
                nc.vector.tensor_tensor(mx, ab[:, :2 * G], ab[:, 2 * G:],
                                        Alu.max)
                nc.vector.tensor_tensor(mn, ab[:, :2 * G], ab[:, 2 * G:],
                                        Alu.min)
                # thr = 2nd-largest |x| per group = max(min(maxes), max(mins))
                t1 = mwork.tile([P, G], bf16, tag="t1")
                t2 = mwork.tile([P, G], bf16, tag="t2")
                nc.vector.tensor_tensor(t1, mx[:, :G], mx[:, G:], Alu.min)
                nc.vector.tensor_tensor(t2, mn[:, :G], mn[:, G:], Alu.max)
                nc.vector.tensor_tensor(t1, t1, t2, Alu.max)
                # prune: xsp = |x| >= thr ? x : 0 (exact bf16 compare)
                xsp = xsp_p.tile([P, D], bf16, tag="xsp")
                thr_b = t1.unsqueeze(1).broadcast_to([P, 4, G])
                nc.vector._custom_dve(
                    PRUNE24,
                    out=xsp.rearrange("p (j g) -> p j g", j=4),
                    in0=xh.rearrange("p (j g) -> p j g", j=4),
                    in1=thr_b)
                # transpose [tok, d] -> [d%128, d//128, tok] on the DMA XBAR,
                # split in two halves on separate queues so dispatch overlaps
                # and the first matmuls only wait on the first half
                xspT = xtp.tile([P, NCH, P], bf16, tag="xspT")
                nc.sync.dma_start(out=xspT[:, :NCH // 2, :],
                                  in_=xsp[:, :D // 2], transpose=True)
                nc.scalar.dma_start(out=xspT[:, NCH // 2:, :],
                                    in_=xsp[:, D // 2:], transpose=True)
                if i == 0:
                    load_w_h1_piece(nc.sync, 0)
                    load_w_h1_piece(nc.scalar, 1)
                elif i == 1:
                    load_w_h1_piece(nc.sync, 2)
                    load_w_h1_piece(nc.scalar, 3)
                # matmul: psum[tok, outf-half] += xspT[c].T @ wT[c]
                for n in range(2):
                    pout = pso.tile([P, OUTF // 2], f32)
                    for c in range(NCH):
                        nc.tensor.matmul(pout,
                                         xspT[:, c, :],
                                         w_sb[:, c, n * 512:(n + 1) * 512],
                                         start=(c == 0), stop=(c == NCH - 1))
                    osb = outp.tile([P, OUTF // 2], f32, tag="osb")
                    nc.scalar.copy(osb, pout)
                    nc.sync.dma_start(
                        out=o_ap[i * P:(i + 1) * P, n * 512:(n + 1) * 512],
                        in_=osb)
    nc.compile()
    return nc


def _get_compiled():
    global _compiled
    if _compiled is None:
        _compiled = _build()
    return _compiled


def _fix_ties_bf16(xb32, x_orig):
    # The device keeps elements with |x| >= (2nd-largest |x| of the group),
    # evaluated on bf16 values.  When the 2nd and 3rd magnitudes round to
    # the same bf16 the device would keep 3+ elements, while the reference
    # (fp32 top_k, stable) keeps exactly 2.  Pre-zero the reference-dropped
    # elements of tied groups so the device selection matches exactly.
    g = np.abs(xb32.reshape(-1, 4))
    s = np.sort(g, axis=1)          # ascending: s[:,2]=2nd largest, s[:,1]=3rd
    tied = s[:, 2] == s[:, 1]
    if not tied.any():
        return xb32
    gv = xb32.reshape(-1, 4)
    go = x_orig.reshape(-1, 4)[tied]
    keep = np.argsort(-np.abs(go), axis=1, kind="stable")[:, :2]
    mask = np.zeros(go.shape, dtype=bool)
    np.put_along_axis(mask, keep, True, axis=1)
    sub = gv[tied]
    sub[~mask] = 0.0
    gv[tied] = sub
    return xb32


# group-position block order: pairs (x0,x1) and (x2,x3) land in opposite
# halves so the DVE tree pairs them with packed stride-1 slices
_BLOCK_ORDER = [0, 2, 1, 3]


def kernel(x: np.ndarray, weight: np.ndarray) -> np.ndarray:
    import ml_dtypes
    from concourse.bass_utils import run_bass_kernel_spmd

    nc = _get_compiled()
    bo = _BLOCK_ORDER
    x_flat = np.ascontiguousarray(x.reshape(TOK_TOTAL, D), dtype=np.float32)
    xb32 = x_flat.astype(ml_dtypes.bfloat16).astype(np.float32)
    xb32 = _fix_ties_bf16(xb32, x_flat)
    # permute d so group position j sits in contiguous block bo.index(j)
    xp = np.ascontiguousarray(
        xb32.reshape(TOK_TOTAL, G, 4)[:, :, bo].transpose(0, 2, 1)
        .reshape(TOK_TOTAL, D)).astype(ml_dtypes.bfloat16)
    wtp = np.ascontiguousarray(
        weight.T.astype(np.float32).reshape(G, 4, OUTF)[:, bo, :]
        .transpose(1, 0, 2).reshape(D, OUTF)).astype(ml_dtypes.bfloat16)
    in_maps = [{"xs": xp[c * TOK:(c + 1) * TOK], "wt": wtp}
               for c in range(N_CORES)]
    res = run_bass_kernel_spmd(nc, in_maps, core_ids=list(range(N_CORES)))
    out = np.concatenate([res.results[c]["o"] for c in range(N_CORES)], axis=0)
    return out.reshape(BS, SEQ, OUTF)
